# revision 24
# baseline (speedup 1.0000x reference)
"""Trainium2 Bass kernel for nn_Attention_87857851006980.

Sharding: 8 cores = 4 batches x 2 head-halves. Core c handles batch c//2,
heads [0..4) (even c) or [4..8) (odd c). Each core computes qkv for its
heads (full-d contraction), the conv/spe branches for its heads' channels,
attention for its heads, and a partial nn1 over its 512 channels; a
pair-wise ReduceScatter then sums the nn1 partials, leaving each core with
half of the output channels for its batch. Host gathers/transposes.

Speed strategy: fp8e4m3 DoubleRow matmuls (0.5 cyc/row, 2x K per pass)
for qkv, conv (with fp8 weight-residual correction packed as the second
K-subtile), q@k dots (d split 64x2 via an SBUF repack DMA), softmax
row-sums and attn@V (j-blocks paired). exp() is computed on big 2-bank
PSUM tiles and split between the Activation engine (Exp -> fp8 out) and
the DVE (Schraudolph: int8 = rint(8*log2e*s + 56) bitcast as e4m3).
nn1 stays bf16. Softmax biases: k needs no bias (constant-per-query terms
cancel in softmax); only q gets bqkv.

All heavy math runs on-device; the host only slices/transposes/quantizes
inputs (layout) and folds BN/bias constants into weight tensors.
"""
import sys
sys.path.insert(0, "/opt/trn_rl_repo")
import numpy as np
import ml_dtypes

import concourse.bacc as bacc
import concourse.bass as bass
import concourse.tile as tile
import concourse.mybir as mybir
from concourse.bass_utils import run_bass_kernel_spmd

F32 = mybir.dt.float32
F32R = mybir.dt.float32r
BF16 = mybir.dt.bfloat16
F8 = mybir.dt.float8e4
I8 = mybir.dt.int8
AF = mybir.ActivationFunctionType
ADD = mybir.AluOpType.add
MULT = mybir.AluOpType.mult
DR = mybir.MatmulPerfMode.DoubleRow
F8NP = ml_dtypes.float8_e4m3

B, D, N, H, HD = 4, 1024, 2304, 8, 128
PS = 48          # image side; N = PS*PS
PP = PS + 2      # padded side
NPAD = PP * PP + 2  # 2502: +2 so the (dy,dx)=(1,1) view of the last row-chunk stays in bounds
HPC = 4          # heads per core
CH = HPC * HD    # 512 channels per core
EPS = 1e-5
SCALE = D ** -0.5
A_SCH = float(8.0 * np.log2(np.e))  # Schraudolph multiplier (pre-scale)

CHUNKS = [(0, 512), (512, 512), (1024, 512), (1536, 512), (2048, 256)]
NJ = N // 128    # 18 key blocks
EB = 8           # qkv output blocks per core: 4 q + 4 k (v folded into wvsum)

USE_COLLECTIVE = True


def _build(single=False):
    use_cc = USE_COLLECTIVE and not single
    nc = bacc.Bacc("TRN2", target_bir_lowering=False, debug=False,
                   num_devices=1 if single else 8)

    # ---- DRAM I/O ----
    x_dn = nc.dram_tensor("x_dn", [D, N], F8, kind="ExternalInput").ap()
    xpad_d = nc.dram_tensor("xpad", [CH, NPAD], F8, kind="ExternalInput").ap()
    wqkv_d = nc.dram_tensor("wqkv", [128, 4 * 2 * EB * 128], F8, kind="ExternalInput").ap()
    wvsum_d = nc.dram_tensor("wvsum", [128, 4 * 2 * 32], F8, kind="ExternalInput").ap()
    wconv_d = nc.dram_tensor("wconv", [128, HPC * 9 * 2 * 128], F8, kind="ExternalInput").ap()
    wnn_d = nc.dram_tensor("wnn", [128, HPC * D], BF16, kind="ExternalInput").ap()
    # small constants batched into one tensor/DMA:
    # [bq 4 | vbias 4 | bn_s 4 | bn_b 4 | bnc_s 4 | bnc_b 4 | bnn1 8 | ones 2 | wspe 512]
    consts_d = nc.dram_tensor("consts", [128, 546], F32, kind="ExternalInput").ap()
    ones8_d = nc.dram_tensor("ones8", [128, 64], F8, kind="ExternalInput").ap()
    identb_d = nc.dram_tensor("identb", [128, 128], BF16, kind="ExternalInput").ap()
    if use_cc:
        out_d = nc.dram_tensor("out", [CH, N], F32, kind="ExternalOutput").ap()
    else:
        out_d = nc.dram_tensor("out", [D, N], F32, kind="ExternalOutput").ap()

    exp_ctr = [0]

    def emit_exp(pt_ap, pd_ap, gw):
        """exp of scores: alternate ACT (exact, fp8 out) and DVE (Schraudolph)."""
        i = exp_ctr[0]
        exp_ctr[0] += 1
        if i % 2 == 0:
            nc.scalar.activation(pt_ap[:, 0:gw], pd_ap[:, 0:gw], AF.Exp, scale=SCALE)
        else:
            nc.vector.tensor_scalar(pt_ap[:, 0:gw].bitcast(I8), pd_ap[:, 0:gw],
                                    A_SCH * SCALE, 56.0, MULT, ADD)

    with tile.TileContext(nc) as tc:
      with tc.tile_pool(name="persist", bufs=1) as pp:
        # ---------- persistent tiles ----------
        qk2 = pp.tile([64, EB * 2 * N], F8, tag="qk2")   # (eb, dhalf, n)
        vpT_sb = pp.tile([128, HPC * NJ * 128], F8, tag="vpT")  # V' = cbr^T + v_spe
        vcol_all = pp.tile([128, HPC * NJ], F32, tag="vcol_all")
        outT_sb = pp.tile([128, HPC * N], BF16, tag="outT")
        wnn_sb = pp.tile([128, HPC * D], BF16, tag="wnn")
        spe_row = pp.tile([1, HPC * 128], F32R, tag="spe_row")
        consts_sb = pp.tile([128, 546], F32, tag="consts")
        ones8_sb = pp.tile([128, 64], F8, tag="ones8")
        ones_row = pp.tile([1, 128], F32R, tag="ones_row")
        identb_sb = pp.tile([128, 128], BF16, tag="identb")
        bq_sb = consts_sb[:, 0:4]
        vbias_sb = consts_sb[:, 4:8]
        bn_s = consts_sb[:, 8:12]
        bn_b = consts_sb[:, 12:16]
        bnc_s = consts_sb[:, 16:20]
        bnc_b = consts_sb[:, 20:24]
        bnn1_sb = consts_sb[:, 24:32]
        ones_c = consts_sb[:, 32:34]
        wspe_sb = pp.tile([128, HPC * 128], F32R, tag="wspe")

        qk2v = qk2[:].rearrange("p (e j two m) -> p e j two m", e=EB, j=NJ, two=2)
        vpTv = vpT_sb[:].rearrange("p (h j c) -> p h j c", h=HPC, j=NJ)

        def load_consts():
            nc.sync.dma_start(consts_sb[:], consts_d[:])
            nc.sync.dma_start(ones8_sb[:], ones8_d[:])
            nc.sync.dma_start(wspe_sb[:], consts_d[:, 34:546].bitcast(F32R))
            nc.sync.dma_start(ones_row[:], ones_c[:, 0:1].bitcast(F32R))
            nc.sync.dma_start(identb_sb[:], identb_d[:])

        # ---- phase A: qkv projection (fp8 DoubleRow, K packed in dt-pairs) ----
        with tc.tile_pool(name="stg", bufs=1) as stg_pool, \
             tc.tile_pool(name="cvin", bufs=4) as cvin_pool, \
             tc.tile_pool(name="cvw", bufs=4) as cvw_pool, \
             tc.tile_pool(name="wqp", bufs=1) as wqp, \
             tc.tile_pool(name="vr", bufs=1) as vr_pool, \
             tc.tile_pool(name="xch", bufs=4) as xch_pool:
          with tc.tile_pool(name="qps", bufs=4, space="PSUM") as qps, \
               tc.tile_pool(name="vps", bufs=2, space="PSUM") as vps:
            stage = stg_pool.tile([128, EB * N], F8)
            wq_t = []
            for dtp in range(4):
                t = wqp.tile([128, 2 * EB * 128], F8, tag=f"wq{dtp}")
                nc.sync.dma_start(
                    t[:], wqkv_d[:, dtp * 2 * EB * 128:(dtp + 1) * 2 * EB * 128])
                wq_t.append(t[:].rearrange("p (e two m) -> p e two m", e=EB, two=2))
            wvs_sb = wqp.tile([128, 4 * 2 * 32], F8)
            nc.sync.dma_start(wvs_sb[:], wvsum_d[:])
            wvsv = wvs_sb[:].rearrange("p (dtp two m) -> p dtp two m", dtp=4, two=2)
            vrow4 = vr_pool.tile([4, N], F32)

            for (n0, nw) in CHUNKS:
                xw = []
                for half in range(2):
                    t = xch_pool.tile([128, 4 * 512], F8, tag="x")
                    src_ap = x_dn[half * 512:(half + 1) * 512, n0:n0 + nw].rearrange(
                        "(t p) n -> p t n", p=128)
                    nc.sync.dma_start(
                        t[:].rearrange("p (t n) -> p t n", t=4)[:, :, 0:nw], src_ap)
                    xw.append(t)
                if n0 == 0:
                    load_consts()
                # dt-pair rhs views: dtp = half*2 + j covers dt (2*dtp, 2*dtp+1)
                xpair = [xw[dtp // 2][:].rearrange("p (t n) -> p t n", t=4)
                         [:, 2 * (dtp % 2):2 * (dtp % 2) + 2, 0:nw]
                         for dtp in range(4)]
                for eb in range(EB):
                    pq = qps.tile([128, 512], F32, tag="q")
                    for dtp in range(4):
                        nc.tensor.matmul(pq[:, 0:nw], wq_t[dtp][:, eb, :, :],
                                         xpair[dtp], start=(dtp == 0),
                                         stop=(dtp == 3), perf_mode=DR)
                    if eb < 4:   # q: add bias, convert to fp8 (ACT)
                        nc.scalar.activation(stage[:, eb * N + n0:eb * N + n0 + nw],
                                             pq[:, 0:nw], AF.Identity,
                                             bias=bq_sb[:, eb:eb + 1])
                    else:        # k: no bias needed (cancels in softmax) (DVE)
                        nc.vector.tensor_copy(stage[:, eb * N + n0:eb * N + n0 + nw],
                                              pq[:, 0:nw])
                pv4 = vps.tile([32, 512], F32, tag="v4")
                for dtp in range(4):
                    nc.tensor.matmul(pv4[:, 0:nw], wvsv[:, dtp, :, :],
                                     xpair[dtp], start=(dtp == 0),
                                     stop=(dtp == 3), perf_mode=DR)
                nc.vector.tensor_copy(vrow4[:, n0:n0 + nw], pv4[0:4, 0:nw])

            # ---- phase B: vsum -> per-head columns via DRAM reshape ----
            with tc.tile_pool(name="vdr", bufs=1, space="DRAM") as vdr_pool:
                vdr = vdr_pool.tile([4, N], F32)
                nc.sync.dma_start(vdr[:], vrow4[:])
                for h in range(HPC):
                    nc.sync.dma_start(
                        vcol_all[:, h * NJ:(h + 1) * NJ],
                        vdr[h:h + 1, :].rearrange("o (j p) -> (o p) j", p=128))
                    nc.vector.tensor_scalar_add(
                        vcol_all[:, h * NJ:(h + 1) * NJ],
                        vcol_all[:, h * NJ:(h + 1) * NJ], vbias_sb[:, h:h + 1])

            # prefetch conv inputs for all heads ahead of the repack DMAs
            xp2s, wcvs = [], []
            for h in range(HPC):
                xp2 = cvin_pool.tile([128, 2 * NPAD], F8, tag="xp")
                xp2v = xp2[:].rearrange("p (two n) -> p two n", two=2)
                nc.sync.dma_start(xp2v[:, 0, :], xpad_d[h * 128:(h + 1) * 128, :])
                nc.sync.dma_start(xp2v[:, 1, :], xpad_d[h * 128:(h + 1) * 128, :])
                wcv = cvw_pool.tile([128, 9 * 2 * 128], F8, tag="wcv")
                nc.sync.dma_start(
                    wcv[:], wconv_d[:, h * 9 * 2 * 128:(h + 1) * 9 * 2 * 128])
                xp2s.append(xp2)
                wcvs.append(wcv)

            # ---- repack q/k: [128, N] -> [64, 2, N] halves for DR dots ----
            for eb in range(EB):
                for half in range(2):
                    nc.sync.dma_start(
                        qk2v[:, eb, :, half, :],
                        stage[half * 64:(half + 1) * 64,
                              eb * N:(eb + 1) * N].rearrange("p (j m) -> p j m", m=128))

          # ---- phase C: conv (fp8 DR with W-residual) + spe; vpT = cbr^T + v_spe ----
          with tc.tile_pool(name="spb", bufs=4) as spb_pool, \
             tc.tile_pool(name="cbr", bufs=2) as cbr_pool, \
             tc.tile_pool(name="scr", bufs=1) as scr_pool, \
             tc.tile_pool(name="pcol", bufs=2) as pcol_pool, \
             tc.tile_pool(name="cps", bufs=2, space="PSUM") as cps, \
             tc.tile_pool(name="tps", bufs=3, space="PSUM") as tps, \
             tc.tile_pool(name="sps", bufs=1, space="PSUM") as sps:
            for h in range(HPC):
                xp2 = xp2s[h]
                xp2v = xp2[:].rearrange("p (two n) -> p two n", two=2)
                wcvv = wcvs[h][:].rearrange("p (o two m) -> p o two m", o=9, two=2)

                # spe branch: gelu(bn(x)) with running row-sum -> pooled -> spe row
                scr = scr_pool.tile([128, N], BF16, tag="scr")
                pcol = pcol_pool.tile([128, 1], F32, tag="pcol")
                interior = xp2[:, PP + 1:PP + 1 + PS * PP].rearrange(
                    "p (r c) -> p r c", c=PP)[:, :, 0:PS]
                nc.scalar.activation(
                    scr[:].rearrange("p (r c) -> p r c", c=PS), interior,
                    AF.Gelu, bias=bn_b[:, h:h + 1], scale=bn_s[:, h:h + 1],
                    accum_out=pcol[:],
                )
                pcol_r = pcol_pool.tile([128, 1], F32R, tag="pcolr")
                nc.vector.tensor_copy(pcol_r[:], pcol[:])
                ps_spe = sps.tile([1, 128], F32, tag="spe")
                nc.tensor.matmul(ps_spe[:], pcol_r[:],
                                 wspe_sb[:, h * 128:(h + 1) * 128], start=True, stop=True)
                nc.vector.tensor_copy(spe_row[:, h * 128:(h + 1) * 128], ps_spe[:])
                # broadcast spe over partitions: spe_bc[p, c] = spe[c]
                ps_bc = sps.tile([128, 128], F32, tag="bc")
                nc.tensor.matmul(ps_bc[:], ones_row[:],
                                 spe_row[:, h * 128:(h + 1) * 128], start=True, stop=True)
                spe_bc = spb_pool.tile([128, 128], F32, tag="spb")
                nc.vector.tensor_copy(spe_bc[:], ps_bc[:])

                # conv branch: 9 offsets as DR pairs (W8, Wr8) over duplicated x
                cbr = cbr_pool.tile([128, N], BF16, tag="cbr")
                for rc in range(6):  # 8 output rows per chunk
                    r0 = rc * 8
                    pc = cps.tile([128, 8 * PS], F32, tag="cv")
                    for oi, (dy, dx) in enumerate(
                            [(a, b) for a in (-1, 0, 1) for b in (-1, 0, 1)]):
                        base = (r0 + 1 + dy) * PP + 1 + dx
                        rhs = xp2v[:, :, base:base + 8 * PP].rearrange(
                            "p two (r c) -> p two r c", c=PP)[:, :, :, 0:PS]
                        nc.tensor.matmul(pc[:].rearrange("p (r c) -> p r c", c=PS),
                                         wcvv[:, oi, :, :], rhs,
                                         start=(oi == 0), stop=(oi == 8),
                                         perf_mode=DR)
                    nc.scalar.activation(cbr[:, r0 * PS:(r0 + 8) * PS], pc[:],
                                         AF.Gelu, bias=bnc_b[:, h:h + 1],
                                         scale=bnc_s[:, h:h + 1])
                # transpose 128-blocks; fold v_spe in the psum->sbuf copy
                for jb in range(NJ):
                    pt_ps = tps.tile([128, 128], BF16, tag="tp")
                    nc.tensor.transpose(pt_ps[:], cbr[:, jb * 128:(jb + 1) * 128],
                                        identb_sb[:])
                    nc.vector.scalar_tensor_tensor(
                        vpTv[:, h, jb, :], spe_bc[:],
                        vcol_all[:, h * NJ + jb:h * NJ + jb + 1],
                        pt_ps[:], MULT, ADD)

        for h in range(HPC):
            nc.sync.dma_start(wnn_sb[:, h * D:(h + 1) * D],
                              wnn_d[:, h * D:(h + 1) * D])

        # ---- phase D+E: attention fused with nn1, chunk-major ----
        # For each query chunk: attention for all 4 heads (fp8 DR dots/sums/AV,
        # exp split ACT/DVE), then the previous chunk's nn1 columns. nn1 and
        # normalize tails fill the PE/ACT/DVE gaps left by the exp pipeline.
        # PSUM: pd ring (2x2 banks, shared by dots/pr/nn1) + po (2) + pm (2).
        with tc.tile_pool(name="ptp", bufs=10) as ptp, \
             tc.tile_pool(name="sums", bufs=2) as sum_pool, \
             tc.tile_pool(name="fin", bufs=3) as fin_pool, \
             tc.tile_pool(name="dram", bufs=1, space="DRAM") as dram, \
             tc.tile_pool(name="dps", bufs=3, space="PSUM") as dps, \
             tc.tile_pool(name="mps", bufs=1, space="PSUM") as mps, \
             tc.tile_pool(name="ops", bufs=1, space="PSUM") as ops:
            if use_cc:
                part = dram.tile([D, N], F32)
                rs0 = dram.tile([CH // 2, N], F32)
                rs1 = dram.tile([CH // 2, N], F32)
            dst = part if use_cc else out_d
            pending = [None]
            workq = []

            def emit_nn1(n0, nw, e0):
                """nn1 columns n0:n0+nw, output blocks e0:e0+2."""
                for ebo in range(e0, e0 + 2):
                    pf = dps.tile([128, 1024], F32, tag="d")
                    for hh in range(HPC):
                        nc.tensor.matmul(
                            pf[:, 0:nw],
                            wnn_sb[:, hh * D + ebo * 128:hh * D + (ebo + 1) * 128],
                            outT_sb[:, hh * N + n0:hh * N + n0 + nw],
                            start=(hh == 0), stop=(hh == HPC - 1))
                    fin = fin_pool.tile([128, 512], F32, tag="fin")
                    nc.scalar.activation(fin[:, 0:nw], pf[:, 0:nw],
                                         AF.Identity, bias=bnn1_sb[:, ebo:ebo + 1])
                    nc.sync.dma_start(
                        dst[ebo * 128:(ebo + 1) * 128, n0:n0 + nw], fin[:, 0:nw])

            dq = []          # global deferred stream: pm/po groups and tails

            def push(f, depth=3):
                dq.append(f)
                while len(dq) > depth:
                    dq.pop(0)()

            for ci, (i0, iw) in enumerate(CHUNKS):
                for h in range(HPC):
                    pm = mps.tile([32, 512], F32, tag="m")
                    po = ops.tile([128, 512], F32, tag="o")
                    per = 1024 // (2 * iw) * 2        # jb per pd tile (2 or 4)
                    groups = []
                    jb0 = 0
                    while jb0 < NJ:
                        groups.append(list(range(jb0, min(jb0 + per, NJ))))
                        jb0 += per

                    def emit_pmpo(pt, npairs, jp0, pm=pm, po=po, h=h, iw=iw):
                        for pi in range(npairs):
                            ptv = pt[:, pi * 2 * iw:(pi + 1) * 2 * iw].rearrange(
                                "p (two n) -> p two n", two=2)
                            jp = jp0 + pi
                            nc.tensor.matmul(
                                pm[:, 0:iw],
                                ones8_sb[:].rearrange("p (two m) -> p two m", m=32),
                                ptv, start=(jp == 0), stop=(jp == NJ // 2 - 1),
                                perf_mode=DR)
                            nc.tensor.matmul(
                                po[:, 0:iw],
                                vpTv[:, h, 2 * jp:2 * jp + 2, :],
                                ptv, start=(jp == 0), stop=(jp == NJ // 2 - 1),
                                perf_mode=DR)

                    def tail(pm=pm, po=po, h=h, i0=i0, iw=iw):
                        rsb = sum_pool.tile([1, 512], F32R, tag="r")
                        with nc.allow_low_precision(
                                reason="f32r keeps full fp32 range"):
                            nc.vector.reciprocal(rsb[:, 0:iw], pm[0:1, 0:iw])
                        pr = dps.tile([128, 1024], F32, tag="d")
                        nc.tensor.matmul(pr[:, 0:iw], ones_row[:], rsb[:, 0:iw],
                                         start=True, stop=True)
                        rbs = sum_pool.tile([128, 512], F32, tag="rbs")
                        nc.scalar.activation(rbs[:, 0:iw], pr[:, 0:iw], AF.Copy)
                        nc.vector.tensor_tensor(
                            outT_sb[:, h * N + i0:h * N + i0 + iw],
                            po[:, 0:iw], rbs[:, 0:iw], MULT)
                        if h == HPC - 1:
                            # chunk complete: queue its nn1 column blocks
                            for eb2 in range(0, 8, 2):
                                workq.append(
                                    lambda e=eb2, n0=i0, nw=iw: emit_nn1(n0, nw, e))

                    jp0 = 0
                    for gidx, grp in enumerate(groups):
                        pd = dps.tile([128, 1024], F32, tag="d")
                        for gi, jb in enumerate(grp):
                            nc.tensor.matmul(
                                pd[:, gi * iw:(gi + 1) * iw],
                                qk2v[:, HPC + h, jb, :, :],
                                qk2v[:, h, i0 // 128:(i0 + iw) // 128, :, :]
                                    .rearrange("p j two m -> p two j m"),
                                start=True, stop=True, perf_mode=DR)
                        pt = ptp.tile([128, 1024], F8, tag="pt")
                        gw = len(grp) * iw
                        emit_exp(pt[:], pd[:], gw)
                        # spread deferred nn1 column blocks across groups,
                        # after the exp so ready exps are not stuck behind
                        # fin copies in the in-order engine streams
                        if workq and gidx % 2 == 1:
                            workq.pop(0)()
                        # pm/po trail the dots by 3 groups, across head and
                        # chunk boundaries; normalize tails ride the same queue
                        npairs = len(grp) // 2
                        push(lambda f=emit_pmpo, pt=pt, np_=npairs, j0=jp0: f(pt, np_, j0))
                        jp0 += npairs
                    push(tail)
            for f in dq:
                f()
            while workq:
                workq.pop(0)()

            if use_cc:
                groups = [[0, 1], [2, 3], [4, 5], [6, 7]]
                nc.gpsimd.collective_compute(
                    "ReduceScatter", ADD, replica_groups=groups,
                    ins=[part[0:CH, :].opt()], outs=[rs0[:].opt()])
                nc.sync.dma_start(out_d[0:CH // 2, :], rs0[:])
                nc.gpsimd.collective_compute(
                    "ReduceScatter", ADD, replica_groups=groups,
                    ins=[part[CH:D, :].opt()], outs=[rs1[:].opt()])
                nc.sync.dma_start(out_d[CH // 2:CH, :], rs1[:])

    nc.compile()
    return nc


def _q8(a):
    return np.asarray(a, np.float32).astype(F8NP)


def _host_inputs(core, inp):
    b, half = core // 2, core % 2
    h0 = half * HPC
    x = np.asarray(inp["x"][b], dtype=np.float32)            # (D, N)
    Wqkv = np.asarray(inp["Wqkv"], dtype=np.float32)
    bqkv = np.asarray(inp["bqkv"], dtype=np.float32)
    Wspe = np.asarray(inp["Wspe"], dtype=np.float32)[:, :, 0, 0]   # (D, H)
    Wlocal = np.asarray(inp["Wlocal"], dtype=np.float32)     # (D, 8, 3, 3)
    Wnn1 = np.asarray(inp["Wnn1"], dtype=np.float32)
    bnn1 = np.asarray(inp["bnn1"], dtype=np.float32)

    chs = slice(h0 * HD, (h0 + HPC) * HD)                    # this core's 512 channels

    # image layout: reinterpret x^T flat as (D, 48, 48); pad to 50x50
    ximg = np.ascontiguousarray(x.T).reshape(D, N)[chs]      # (512, 2304)
    pad = np.zeros((CH, NPAD), np.float32)
    pad3 = pad[:, :PP * PP].reshape(CH, PP, PP)
    pad3[:, 1:PS + 1, 1:PS + 1] = ximg.reshape(CH, PS, PS)
    xpad = _q8(pad)

    # qkv weights: e-blocks = [q heads, k heads], DR dt-pair lhsT layout
    rows = np.concatenate(
        [np.arange(h0 * HD, (h0 + HPC) * HD) + s * D for s in range(2)])
    wqkvT = Wqkv[rows, :].T                                   # (1024 d_in, 1024 out)
    wq = wqkvT.reshape(4, 2, 128, EB, 128).transpose(2, 0, 3, 1, 4).reshape(
        128, 4 * 2 * EB * 128)
    bq = bqkv[np.arange(h0 * HD, (h0 + HPC) * HD)].reshape(HPC, 128).T.copy()
    vrows = np.arange(h0 * HD, (h0 + HPC) * HD) + 2 * D
    wv = Wqkv[vrows, :].reshape(HPC, 128, D).sum(axis=1)      # (HPC, 1024)
    wvsum4 = wv.T.reshape(4, 2, 128, HPC).transpose(2, 0, 1, 3)  # (128, 4, 2, 4)
    wvsum = np.zeros((128, 4, 2, 32), np.float32)
    wvsum[:, :, :, 0:HPC] = wvsum4
    wvsum = wvsum.reshape(128, 4 * 2 * 32)
    vb = bqkv[vrows].reshape(HPC, 128).sum(axis=1)            # summed v bias per head
    vbias = np.repeat(vb[None, :], 128, axis=0).astype(np.float32)

    # dense per-head conv weights, DR pairs (W8, Wr8) per (head, offset)
    wconv = np.zeros((HPC, 9, 128, 128), np.float32)
    for h in range(HPC):
        for co in range(128):
            g = co // 8
            cg = np.arange(g * 8, g * 8 + 8)
            for oi, (dy, dx) in enumerate(
                    [(a, c) for a in range(3) for c in range(3)]):
                wconv[h, oi, cg, co] = Wlocal[(h0 + h) * HD + co, :, dy, dx]
    w8 = _q8(wconv)
    wr8 = _q8(wconv - w8.astype(np.float32))
    wpair = np.stack([w8, wr8], axis=2)                       # (HPC, 9, 2, 128, 128)
    wconv8 = wpair.transpose(3, 0, 1, 2, 4).reshape(128, HPC * 9 * 2 * 128)

    # spe block-diag matrix (folds in 1/N pooling mean and attention scale)
    wspe = np.zeros((HPC, 128, 128), np.float32)              # [h, c_in, idx]
    for h in range(HPC):
        for gg in range(16):
            g = (h0 + h) * 16 + gg
            blk = Wspe[g * 8:(g + 1) * 8, :8]                 # [o, i]
            wspe[h, gg * 8:gg * 8 + 8, gg * 8:gg * 8 + 8] = blk.T  # [i, o]
    wspe = (wspe * (SCALE / N)).transpose(1, 0, 2).reshape(128, HPC * 128)

    def fold_bn(g, bta, mu, var):
        s = np.asarray(g, np.float64) / np.sqrt(np.asarray(var, np.float64) + EPS)
        return (s.astype(np.float32),
                (np.asarray(bta, np.float64) - np.asarray(mu, np.float64) * s)
                .astype(np.float32))

    bn_s, bn_b = fold_bn(inp["bn_gamma"], inp["bn_beta"], inp["bn_mean"], inp["bn_var"])
    bnc_s, bnc_b = fold_bn(inp["bnc_gamma"], inp["bnc_beta"], inp["bnc_mean"],
                           inp["bnc_var"])
    shp = lambda a: np.ascontiguousarray(a[chs].reshape(HPC, 128).T)

    wnn1T = Wnn1[:, chs].T                                    # (512, 1024)
    wnn1 = wnn1T.reshape(HPC, 128, D).transpose(1, 0, 2).reshape(128, HPC * D)
    bnn1h = np.ascontiguousarray((0.5 * bnn1).reshape(8, 128).T)

    consts = np.concatenate([
        bq, vbias, shp(bn_s), shp(bn_b), shp(bnc_s), shp(bnc_b),
        bnn1h, np.ones((128, 2), np.float32), wspe], axis=1).astype(np.float32)
    return {
        "x_dn": _q8(x), "xpad": xpad,
        "wqkv": np.ascontiguousarray(wq.astype(F8NP)),
        "wvsum": np.ascontiguousarray(wvsum.astype(F8NP)),
        "wconv": np.ascontiguousarray(wconv8),
        "wnn": np.ascontiguousarray(wnn1.astype(ml_dtypes.bfloat16)),
        "consts": np.ascontiguousarray(consts),
        "ones8": np.ones((128, 64), F8NP),
        "identb": np.eye(128, dtype=ml_dtypes.bfloat16),
    }


_NC = None


def kernel(**inputs):
    global _NC
    if _NC is None:
        _NC = _build()
    in_maps = [_host_inputs(c, inputs) for c in range(8)]
    res = run_bass_kernel_spmd(_NC, in_maps, core_ids=list(range(8)))
    out = np.empty((B, N, D), np.float32)
    for b in range(B):
        if USE_COLLECTIVE:
            ev, od = res.results[2 * b]["out"], res.results[2 * b + 1]["out"]
            t = np.empty((D, N), np.float32)
            t[0:256] = ev[0:256]
            t[256:512] = od[0:256]
            t[512:768] = ev[256:512]
            t[768:1024] = od[256:512]
        else:
            t = res.results[2 * b]["out"] + res.results[2 * b + 1]["out"]
        out[b] = t.T
    return out


def run_timed(**inputs):
    """Re-run with NTFF tracing to get HW exec time (best effort)."""
    global _NC
    if _NC is None:
        _NC = _build()
    in_maps = [_host_inputs(c, inputs) for c in range(8)]
    try:
        return run_bass_kernel_spmd(_NC, in_maps, core_ids=list(range(8)), trace=True)
    except Exception as e:  # tracing unsupported under some axon terminals
        print(f"trace run failed: {e}")
        return None


# revision 25
# speedup vs baseline: 1.0543x; 1.0543x over previous
"""Trainium2 Bass kernel for nn_Attention_87857851006980.

Sharding: 8 cores = 4 batches x 2 head-halves. Core c handles batch c//2,
heads [0..4) (even c) or [4..8) (odd c). Each core computes qkv for its
heads (full-d contraction), the conv/spe branches for its heads' channels,
attention for its heads, and a partial nn1 over its 512 channels; a
pair-wise ReduceScatter then sums the nn1 partials, leaving each core with
half of the output channels for its batch. Host gathers/transposes.

Speed strategy: fp8e4m3 DoubleRow matmuls (0.5 cyc/row, 2x K per pass)
for qkv, conv (with fp8 weight-residual correction packed as the second
K-subtile), q@k dots (d split 64x2 via an SBUF repack DMA), softmax
row-sums and attn@V (j-blocks paired). exp() is computed on big 2-bank
PSUM tiles and split between the Activation engine (Exp -> fp8 out) and
the DVE (Schraudolph: int8 = rint(8*log2e*s + 56) bitcast as e4m3).
nn1 stays bf16. Softmax biases: k needs no bias (constant-per-query terms
cancel in softmax); only q gets bqkv.

All heavy math runs on-device; the host only slices/transposes/quantizes
inputs (layout) and folds BN/bias constants into weight tensors.
"""
import sys
sys.path.insert(0, "/opt/trn_rl_repo")
import numpy as np
import ml_dtypes

import concourse.bacc as bacc
import concourse.bass as bass
import concourse.tile as tile
import concourse.mybir as mybir
from concourse.bass_utils import run_bass_kernel_spmd

F32 = mybir.dt.float32
F32R = mybir.dt.float32r
BF16 = mybir.dt.bfloat16
F8 = mybir.dt.float8e4
I8 = mybir.dt.int8
AF = mybir.ActivationFunctionType
ADD = mybir.AluOpType.add
MULT = mybir.AluOpType.mult
DR = mybir.MatmulPerfMode.DoubleRow
F8NP = ml_dtypes.float8_e4m3

B, D, N, H, HD = 4, 1024, 2304, 8, 128
PS = 48          # image side; N = PS*PS
PP = PS + 2      # padded side
NPAD = PP * PP + 2  # 2502: +2 so the (dy,dx)=(1,1) view of the last row-chunk stays in bounds
HPC = 4          # heads per core
CH = HPC * HD    # 512 channels per core
EPS = 1e-5
SCALE = D ** -0.5
A_SCH = float(8.0 * np.log2(np.e))  # Schraudolph multiplier (pre-scale)

CHUNKS = [(0, 512), (512, 512), (1024, 512), (1536, 512), (2048, 256)]
NJ = N // 128    # 18 key blocks
EB = 8           # qkv output blocks per core: 4 q + 4 k (v folded into wvsum)

USE_COLLECTIVE = True


def _build(single=False):
    use_cc = USE_COLLECTIVE and not single
    nc = bacc.Bacc("TRN2", target_bir_lowering=False, debug=False,
                   num_devices=1 if single else 8)

    # ---- DRAM I/O ----
    x_dn = nc.dram_tensor("x_dn", [D, N], F8, kind="ExternalInput").ap()
    xpad_d = nc.dram_tensor("xpad", [CH, NPAD], F8, kind="ExternalInput").ap()
    wqkv_d = nc.dram_tensor("wqkv", [128, 4 * 2 * EB * 128], F8, kind="ExternalInput").ap()
    wvsum_d = nc.dram_tensor("wvsum", [128, 4 * 2 * 32], F8, kind="ExternalInput").ap()
    wconv_d = nc.dram_tensor("wconv", [128, HPC * 9 * 2 * 128], F8, kind="ExternalInput").ap()
    wnn_d = nc.dram_tensor("wnn", [128, HPC * D], BF16, kind="ExternalInput").ap()
    # small constants batched into one tensor/DMA:
    # [bq 4 | vbias 4 | bn_s 4 | bn_b 4 | bnc_s 4 | bnc_b 4 | bnn1 8 | ones 2 | wspe 512]
    consts_d = nc.dram_tensor("consts", [128, 546], F32, kind="ExternalInput").ap()
    ones8_d = nc.dram_tensor("ones8", [128, 64], F8, kind="ExternalInput").ap()
    identb_d = nc.dram_tensor("identb", [128, 128], BF16, kind="ExternalInput").ap()
    if use_cc:
        out_d = nc.dram_tensor("out", [CH, N], F32, kind="ExternalOutput").ap()
    else:
        out_d = nc.dram_tensor("out", [D, N], F32, kind="ExternalOutput").ap()

    exp_ctr = [0]

    def emit_exp(pt_ap, pd_ap, gw):
        """exp of scores: alternate ACT (exact, fp8 out) and DVE (Schraudolph)."""
        i = exp_ctr[0]
        exp_ctr[0] += 1
        if i % 2 == 0:
            nc.scalar.activation(pt_ap[:, 0:gw], pd_ap[:, 0:gw], AF.Exp, scale=SCALE)
        else:
            nc.vector.tensor_scalar(pt_ap[:, 0:gw].bitcast(I8), pd_ap[:, 0:gw],
                                    A_SCH * SCALE, 56.0, MULT, ADD)

    with tile.TileContext(nc) as tc:
      with tc.tile_pool(name="persist", bufs=1) as pp:
        # ---------- persistent tiles ----------
        qk2 = pp.tile([64, EB * 2 * N], F8, tag="qk2")   # (eb, dhalf, n)
        vpT_sb = pp.tile([128, HPC * NJ * 128], F8, tag="vpT")  # V' = cbr^T + v_spe
        vcol_all = pp.tile([128, HPC * NJ], F32, tag="vcol_all")
        outT_sb = pp.tile([128, HPC * N], BF16, tag="outT")
        wnn_sb = pp.tile([128, HPC * D], BF16, tag="wnn")
        spe_row = pp.tile([1, HPC * 128], F32R, tag="spe_row")
        consts_sb = pp.tile([128, 546], F32, tag="consts")
        ones8_sb = pp.tile([128, 64], F8, tag="ones8")
        ones_row = pp.tile([1, 128], F32R, tag="ones_row")
        identb_sb = pp.tile([128, 128], BF16, tag="identb")
        bq_sb = consts_sb[:, 0:4]
        vbias_sb = consts_sb[:, 4:8]
        bn_s = consts_sb[:, 8:12]
        bn_b = consts_sb[:, 12:16]
        bnc_s = consts_sb[:, 16:20]
        bnc_b = consts_sb[:, 20:24]
        bnn1_sb = consts_sb[:, 24:32]
        ones_c = consts_sb[:, 32:34]
        wspe_sb = pp.tile([128, HPC * 128], F32R, tag="wspe")

        qk2v = qk2[:].rearrange("p (e j two m) -> p e j two m", e=EB, j=NJ, two=2)
        vpTv = vpT_sb[:].rearrange("p (h j c) -> p h j c", h=HPC, j=NJ)

        def load_consts():
            nc.sync.dma_start(consts_sb[:], consts_d[:])
            nc.sync.dma_start(ones8_sb[:], ones8_d[:])
            nc.sync.dma_start(wspe_sb[:], consts_d[:, 34:546].bitcast(F32R))
            nc.sync.dma_start(ones_row[:], ones_c[:, 0:1].bitcast(F32R))
            nc.sync.dma_start(identb_sb[:], identb_d[:])

        # ---- phase A: qkv projection (fp8 DoubleRow, K packed in dt-pairs) ----
        with tc.tile_pool(name="stg", bufs=1) as stg_pool, \
             tc.tile_pool(name="cvin", bufs=4) as cvin_pool, \
             tc.tile_pool(name="cvw", bufs=4) as cvw_pool, \
             tc.tile_pool(name="wqp", bufs=1) as wqp, \
             tc.tile_pool(name="vr", bufs=1) as vr_pool, \
             tc.tile_pool(name="xch", bufs=4) as xch_pool:
          with tc.tile_pool(name="qps", bufs=4, space="PSUM") as qps, \
               tc.tile_pool(name="vps", bufs=2, space="PSUM") as vps:
            stage = stg_pool.tile([128, EB * N], F8)
            wq_t = []
            for dtp in range(4):
                t = wqp.tile([128, 2 * EB * 128], F8, tag=f"wq{dtp}")
                nc.sync.dma_start(
                    t[:], wqkv_d[:, dtp * 2 * EB * 128:(dtp + 1) * 2 * EB * 128])
                wq_t.append(t[:].rearrange("p (e two m) -> p e two m", e=EB, two=2))
            wvs_sb = wqp.tile([128, 4 * 2 * 32], F8)
            nc.sync.dma_start(wvs_sb[:], wvsum_d[:])
            wvsv = wvs_sb[:].rearrange("p (dtp two m) -> p dtp two m", dtp=4, two=2)
            vrow4 = vr_pool.tile([4, N], F32)

            for (n0, nw) in CHUNKS:
                xw = []
                for half in range(2):
                    t = xch_pool.tile([128, 4 * 512], F8, tag="x")
                    src_ap = x_dn[half * 512:(half + 1) * 512, n0:n0 + nw].rearrange(
                        "(t p) n -> p t n", p=128)
                    nc.sync.dma_start(
                        t[:].rearrange("p (t n) -> p t n", t=4)[:, :, 0:nw], src_ap)
                    xw.append(t)
                if n0 == 0:
                    load_consts()
                # dt-pair rhs views: dtp = half*2 + j covers dt (2*dtp, 2*dtp+1)
                xpair = [xw[dtp // 2][:].rearrange("p (t n) -> p t n", t=4)
                         [:, 2 * (dtp % 2):2 * (dtp % 2) + 2, 0:nw]
                         for dtp in range(4)]
                for eb in range(EB):
                    pq = qps.tile([128, 512], F32, tag="q")
                    for dtp in range(4):
                        nc.tensor.matmul(pq[:, 0:nw], wq_t[dtp][:, eb, :, :],
                                         xpair[dtp], start=(dtp == 0),
                                         stop=(dtp == 3), perf_mode=DR)
                    if eb < 4:   # q: add bias, convert to fp8 (ACT)
                        nc.scalar.activation(stage[:, eb * N + n0:eb * N + n0 + nw],
                                             pq[:, 0:nw], AF.Identity,
                                             bias=bq_sb[:, eb:eb + 1])
                    else:        # k: no bias needed (cancels in softmax) (DVE)
                        nc.vector.tensor_copy(stage[:, eb * N + n0:eb * N + n0 + nw],
                                              pq[:, 0:nw])
                pv4 = vps.tile([32, 512], F32, tag="v4")
                for dtp in range(4):
                    nc.tensor.matmul(pv4[:, 0:nw], wvsv[:, dtp, :, :],
                                     xpair[dtp], start=(dtp == 0),
                                     stop=(dtp == 3), perf_mode=DR)
                nc.vector.tensor_copy(vrow4[:, n0:n0 + nw], pv4[0:4, 0:nw])

            # ---- phase B: vsum -> per-head columns via DRAM reshape ----
            with tc.tile_pool(name="vdr", bufs=1, space="DRAM") as vdr_pool:
                vdr = vdr_pool.tile([4, N], F32)
                nc.sync.dma_start(vdr[:], vrow4[:])
                for h in range(HPC):
                    nc.sync.dma_start(
                        vcol_all[:, h * NJ:(h + 1) * NJ],
                        vdr[h:h + 1, :].rearrange("o (j p) -> (o p) j", p=128))
                    nc.vector.tensor_scalar_add(
                        vcol_all[:, h * NJ:(h + 1) * NJ],
                        vcol_all[:, h * NJ:(h + 1) * NJ], vbias_sb[:, h:h + 1])

            # prefetch conv inputs for all heads ahead of the repack DMAs
            xp2s, wcvs = [], []
            for h in range(HPC):
                xp2 = cvin_pool.tile([128, 2 * NPAD], F8, tag="xp")
                xp2v = xp2[:].rearrange("p (two n) -> p two n", two=2)
                nc.sync.dma_start(xp2v[:, 0, :], xpad_d[h * 128:(h + 1) * 128, :])
                nc.sync.dma_start(xp2v[:, 1, :], xpad_d[h * 128:(h + 1) * 128, :])
                wcv = cvw_pool.tile([128, 9 * 2 * 128], F8, tag="wcv")
                nc.sync.dma_start(
                    wcv[:], wconv_d[:, h * 9 * 2 * 128:(h + 1) * 9 * 2 * 128])
                xp2s.append(xp2)
                wcvs.append(wcv)

            # ---- repack q/k: [128, N] -> [64, 2, N] halves for DR dots ----
            for eb in range(EB):
                for half in range(2):
                    nc.sync.dma_start(
                        qk2v[:, eb, :, half, :],
                        stage[half * 64:(half + 1) * 64,
                              eb * N:(eb + 1) * N].rearrange("p (j m) -> p j m", m=128))

          # ---- phase C: conv (fp8 DR with W-residual) + spe; vpT = cbr^T + v_spe ----
          with tc.tile_pool(name="spb", bufs=4) as spb_pool, \
             tc.tile_pool(name="cbr", bufs=2) as cbr_pool, \
             tc.tile_pool(name="scr", bufs=1) as scr_pool, \
             tc.tile_pool(name="pcol", bufs=2) as pcol_pool, \
             tc.tile_pool(name="cps", bufs=2, space="PSUM") as cps, \
             tc.tile_pool(name="tps", bufs=3, space="PSUM") as tps, \
             tc.tile_pool(name="sps", bufs=1, space="PSUM") as sps:
            for h in range(HPC):
                xp2 = xp2s[h]
                xp2v = xp2[:].rearrange("p (two n) -> p two n", two=2)
                wcvv = wcvs[h][:].rearrange("p (o two m) -> p o two m", o=9, two=2)

                # spe branch: gelu(bn(x)) with running row-sum -> pooled -> spe row
                scr = scr_pool.tile([128, N], BF16, tag="scr")
                pcol = pcol_pool.tile([128, 1], F32, tag="pcol")
                interior = xp2[:, PP + 1:PP + 1 + PS * PP].rearrange(
                    "p (r c) -> p r c", c=PP)[:, :, 0:PS]
                nc.scalar.activation(
                    scr[:].rearrange("p (r c) -> p r c", c=PS), interior,
                    AF.Gelu, bias=bn_b[:, h:h + 1], scale=bn_s[:, h:h + 1],
                    accum_out=pcol[:],
                )
                pcol_r = pcol_pool.tile([128, 1], F32R, tag="pcolr")
                nc.vector.tensor_copy(pcol_r[:], pcol[:])
                ps_spe = sps.tile([1, 128], F32, tag="spe")
                nc.tensor.matmul(ps_spe[:], pcol_r[:],
                                 wspe_sb[:, h * 128:(h + 1) * 128], start=True, stop=True)
                nc.vector.tensor_copy(spe_row[:, h * 128:(h + 1) * 128], ps_spe[:])
                # broadcast spe over partitions: spe_bc[p, c] = spe[c]
                ps_bc = sps.tile([128, 128], F32, tag="bc")
                nc.tensor.matmul(ps_bc[:], ones_row[:],
                                 spe_row[:, h * 128:(h + 1) * 128], start=True, stop=True)
                spe_bc = spb_pool.tile([128, 128], F32, tag="spb")
                nc.vector.tensor_copy(spe_bc[:], ps_bc[:])

                # conv branch: 9 offsets as DR pairs (W8, Wr8) over duplicated x
                cbr = cbr_pool.tile([128, N], BF16, tag="cbr")
                for rc in range(6):  # 8 output rows per chunk
                    r0 = rc * 8
                    pc = cps.tile([128, 8 * PS], F32, tag="cv")
                    for oi, (dy, dx) in enumerate(
                            [(a, b) for a in (-1, 0, 1) for b in (-1, 0, 1)]):
                        base = (r0 + 1 + dy) * PP + 1 + dx
                        rhs = xp2v[:, :, base:base + 8 * PP].rearrange(
                            "p two (r c) -> p two r c", c=PP)[:, :, :, 0:PS]
                        nc.tensor.matmul(pc[:].rearrange("p (r c) -> p r c", c=PS),
                                         wcvv[:, oi, :, :], rhs,
                                         start=(oi == 0), stop=(oi == 8),
                                         perf_mode=DR)
                    nc.scalar.activation(cbr[:, r0 * PS:(r0 + 8) * PS], pc[:],
                                         AF.Gelu, bias=bnc_b[:, h:h + 1],
                                         scale=bnc_s[:, h:h + 1])
                # transpose 128-blocks; fold v_spe in the psum->sbuf copy
                for jb in range(NJ):
                    pt_ps = tps.tile([128, 128], BF16, tag="tp")
                    nc.tensor.transpose(pt_ps[:], cbr[:, jb * 128:(jb + 1) * 128],
                                        identb_sb[:])
                    nc.vector.scalar_tensor_tensor(
                        vpTv[:, h, jb, :], spe_bc[:],
                        vcol_all[:, h * NJ + jb:h * NJ + jb + 1],
                        pt_ps[:], MULT, ADD)

        for h in range(HPC):
            nc.sync.dma_start(wnn_sb[:, h * D:(h + 1) * D],
                              wnn_d[:, h * D:(h + 1) * D])

        # ---- phase D+E: attention fused with nn1, chunk-major ----
        # For each query chunk: attention for all 4 heads (fp8 DR dots/sums/AV,
        # exp split ACT/DVE), then the previous chunk's nn1 columns. nn1 and
        # normalize tails fill the PE/ACT/DVE gaps left by the exp pipeline.
        # PSUM: pd ring (2x2 banks, shared by dots/pr/nn1) + po (2) + pm (2).
        with tc.tile_pool(name="ptp", bufs=10) as ptp, \
             tc.tile_pool(name="sums", bufs=2) as sum_pool, \
             tc.tile_pool(name="fin", bufs=3) as fin_pool, \
             tc.tile_pool(name="dram", bufs=1, space="DRAM") as dram, \
             tc.tile_pool(name="dps", bufs=3, space="PSUM") as dps, \
             tc.tile_pool(name="mps", bufs=1, space="PSUM") as mps, \
             tc.tile_pool(name="ops", bufs=1, space="PSUM") as ops:
            if use_cc:
                part = dram.tile([D, N], F32)
                rs0 = dram.tile([CH // 2, N], F32)
                rs1 = dram.tile([CH // 2, N], F32)
            dst = part if use_cc else out_d
            pending = [None]
            workq = []

            def emit_nn1(n0, nw, e0):
                """nn1 columns n0:n0+nw, output blocks e0:e0+2."""
                for ebo in range(e0, e0 + 2):
                    pf = dps.tile([128, 1024], F32, tag="d")
                    for hh in range(HPC):
                        nc.tensor.matmul(
                            pf[:, 0:nw],
                            wnn_sb[:, hh * D + ebo * 128:hh * D + (ebo + 1) * 128],
                            outT_sb[:, hh * N + n0:hh * N + n0 + nw],
                            start=(hh == 0), stop=(hh == HPC - 1))
                    fin = fin_pool.tile([128, 512], F32, tag="fin")
                    nc.scalar.activation(fin[:, 0:nw], pf[:, 0:nw],
                                         AF.Identity, bias=bnn1_sb[:, ebo:ebo + 1])
                    nc.sync.dma_start(
                        dst[ebo * 128:(ebo + 1) * 128, n0:n0 + nw], fin[:, 0:nw])

            dq = []          # global deferred stream: pm/po groups and tails

            def push(f, depth=4):
                dq.append(f)
                while len(dq) > depth:
                    dq.pop(0)()

            for ci, (i0, iw) in enumerate(CHUNKS):
                for h in range(HPC):
                    pm = mps.tile([32, 512], F32, tag="m")
                    po = ops.tile([128, 512], F32, tag="o")
                    per = 1024 // (2 * iw) * 2        # jb per pd tile (2 or 4)
                    groups = []
                    jb0 = 0
                    while jb0 < NJ:
                        groups.append(list(range(jb0, min(jb0 + per, NJ))))
                        jb0 += per

                    def emit_pmpo(pt, npairs, jp0, pm=pm, po=po, h=h, iw=iw):
                        for pi in range(npairs):
                            ptv = pt[:, pi * 2 * iw:(pi + 1) * 2 * iw].rearrange(
                                "p (two n) -> p two n", two=2)
                            jp = jp0 + pi
                            nc.tensor.matmul(
                                pm[:, 0:iw],
                                ones8_sb[:].rearrange("p (two m) -> p two m", m=32),
                                ptv, start=(jp == 0), stop=(jp == NJ // 2 - 1),
                                perf_mode=DR)
                            nc.tensor.matmul(
                                po[:, 0:iw],
                                vpTv[:, h, 2 * jp:2 * jp + 2, :],
                                ptv, start=(jp == 0), stop=(jp == NJ // 2 - 1),
                                perf_mode=DR)

                    rsb_box = [None]

                    def tail_a(pm=pm, iw=iw, rsb_box=rsb_box):
                        # reciprocal alone: lands on DVE right after pm-stop,
                        # ahead of the next exps
                        rsb = sum_pool.tile([1, 512], F32R, tag="r")
                        with nc.allow_low_precision(
                                reason="f32r keeps full fp32 range"):
                            nc.vector.reciprocal(rsb[:, 0:iw], pm[0:1, 0:iw])
                        rsb_box[0] = rsb

                    def tail_b(po=po, h=h, i0=i0, iw=iw, rsb_box=rsb_box):
                        rsb = rsb_box[0]
                        pr = dps.tile([128, 1024], F32, tag="d")
                        nc.tensor.matmul(pr[:, 0:iw], ones_row[:], rsb[:, 0:iw],
                                         start=True, stop=True)
                        rbs = sum_pool.tile([128, 512], F32, tag="rbs")
                        nc.scalar.activation(rbs[:, 0:iw], pr[:, 0:iw], AF.Copy)
                        nc.vector.tensor_tensor(
                            outT_sb[:, h * N + i0:h * N + i0 + iw],
                            po[:, 0:iw], rbs[:, 0:iw], MULT)
                        if h == HPC - 1:
                            # chunk complete: queue its nn1 column blocks
                            for eb2 in range(0, 8, 2):
                                workq.append(
                                    lambda e=eb2, n0=i0, nw=iw: emit_nn1(n0, nw, e))

                    jp0 = 0
                    for gidx, grp in enumerate(groups):
                        pd = dps.tile([128, 1024], F32, tag="d")
                        for gi, jb in enumerate(grp):
                            nc.tensor.matmul(
                                pd[:, gi * iw:(gi + 1) * iw],
                                qk2v[:, HPC + h, jb, :, :],
                                qk2v[:, h, i0 // 128:(i0 + iw) // 128, :, :]
                                    .rearrange("p j two m -> p two j m"),
                                start=True, stop=True, perf_mode=DR)
                        pt = ptp.tile([128, 1024], F8, tag="pt")
                        gw = len(grp) * iw
                        emit_exp(pt[:], pd[:], gw)
                        # spread deferred nn1 column blocks across groups,
                        # after the exp so ready exps are not stuck behind
                        # fin copies in the in-order engine streams
                        if workq and gidx % 2 == 1:
                            workq.pop(0)()
                        # pm/po trail the dots by 3 groups, across head and
                        # chunk boundaries; normalize tails ride the same queue
                        npairs = len(grp) // 2
                        push(lambda f=emit_pmpo, pt=pt, np_=npairs, j0=jp0: f(pt, np_, j0))
                        jp0 += npairs
                    push(tail_a)
                    push(tail_b)
            for f in dq:
                f()
            while workq:
                workq.pop(0)()

            if use_cc:
                groups = [[0, 1], [2, 3], [4, 5], [6, 7]]
                nc.gpsimd.collective_compute(
                    "ReduceScatter", ADD, replica_groups=groups,
                    ins=[part[0:CH, :].opt()], outs=[rs0[:].opt()])
                nc.sync.dma_start(out_d[0:CH // 2, :], rs0[:])
                nc.gpsimd.collective_compute(
                    "ReduceScatter", ADD, replica_groups=groups,
                    ins=[part[CH:D, :].opt()], outs=[rs1[:].opt()])
                nc.sync.dma_start(out_d[CH // 2:CH, :], rs1[:])

    nc.compile()
    return nc


def _q8(a):
    return np.asarray(a, np.float32).astype(F8NP)


def _host_inputs(core, inp):
    b, half = core // 2, core % 2
    h0 = half * HPC
    x = np.asarray(inp["x"][b], dtype=np.float32)            # (D, N)
    Wqkv = np.asarray(inp["Wqkv"], dtype=np.float32)
    bqkv = np.asarray(inp["bqkv"], dtype=np.float32)
    Wspe = np.asarray(inp["Wspe"], dtype=np.float32)[:, :, 0, 0]   # (D, H)
    Wlocal = np.asarray(inp["Wlocal"], dtype=np.float32)     # (D, 8, 3, 3)
    Wnn1 = np.asarray(inp["Wnn1"], dtype=np.float32)
    bnn1 = np.asarray(inp["bnn1"], dtype=np.float32)

    chs = slice(h0 * HD, (h0 + HPC) * HD)                    # this core's 512 channels

    # image layout: reinterpret x^T flat as (D, 48, 48); pad to 50x50
    ximg = np.ascontiguousarray(x.T).reshape(D, N)[chs]      # (512, 2304)
    pad = np.zeros((CH, NPAD), np.float32)
    pad3 = pad[:, :PP * PP].reshape(CH, PP, PP)
    pad3[:, 1:PS + 1, 1:PS + 1] = ximg.reshape(CH, PS, PS)
    xpad = _q8(pad)

    # qkv weights: e-blocks = [q heads, k heads], DR dt-pair lhsT layout
    rows = np.concatenate(
        [np.arange(h0 * HD, (h0 + HPC) * HD) + s * D for s in range(2)])
    wqkvT = Wqkv[rows, :].T                                   # (1024 d_in, 1024 out)
    wq = wqkvT.reshape(4, 2, 128, EB, 128).transpose(2, 0, 3, 1, 4).reshape(
        128, 4 * 2 * EB * 128)
    bq = bqkv[np.arange(h0 * HD, (h0 + HPC) * HD)].reshape(HPC, 128).T.copy()
    vrows = np.arange(h0 * HD, (h0 + HPC) * HD) + 2 * D
    wv = Wqkv[vrows, :].reshape(HPC, 128, D).sum(axis=1)      # (HPC, 1024)
    wvsum4 = wv.T.reshape(4, 2, 128, HPC).transpose(2, 0, 1, 3)  # (128, 4, 2, 4)
    wvsum = np.zeros((128, 4, 2, 32), np.float32)
    wvsum[:, :, :, 0:HPC] = wvsum4
    wvsum = wvsum.reshape(128, 4 * 2 * 32)
    vb = bqkv[vrows].reshape(HPC, 128).sum(axis=1)            # summed v bias per head
    vbias = np.repeat(vb[None, :], 128, axis=0).astype(np.float32)

    # dense per-head conv weights, DR pairs (W8, Wr8) per (head, offset)
    wconv = np.zeros((HPC, 9, 128, 128), np.float32)
    for h in range(HPC):
        for co in range(128):
            g = co // 8
            cg = np.arange(g * 8, g * 8 + 8)
            for oi, (dy, dx) in enumerate(
                    [(a, c) for a in range(3) for c in range(3)]):
                wconv[h, oi, cg, co] = Wlocal[(h0 + h) * HD + co, :, dy, dx]
    w8 = _q8(wconv)
    wr8 = _q8(wconv - w8.astype(np.float32))
    wpair = np.stack([w8, wr8], axis=2)                       # (HPC, 9, 2, 128, 128)
    wconv8 = wpair.transpose(3, 0, 1, 2, 4).reshape(128, HPC * 9 * 2 * 128)

    # spe block-diag matrix (folds in 1/N pooling mean and attention scale)
    wspe = np.zeros((HPC, 128, 128), np.float32)              # [h, c_in, idx]
    for h in range(HPC):
        for gg in range(16):
            g = (h0 + h) * 16 + gg
            blk = Wspe[g * 8:(g + 1) * 8, :8]                 # [o, i]
            wspe[h, gg * 8:gg * 8 + 8, gg * 8:gg * 8 + 8] = blk.T  # [i, o]
    wspe = (wspe * (SCALE / N)).transpose(1, 0, 2).reshape(128, HPC * 128)

    def fold_bn(g, bta, mu, var):
        s = np.asarray(g, np.float64) / np.sqrt(np.asarray(var, np.float64) + EPS)
        return (s.astype(np.float32),
                (np.asarray(bta, np.float64) - np.asarray(mu, np.float64) * s)
                .astype(np.float32))

    bn_s, bn_b = fold_bn(inp["bn_gamma"], inp["bn_beta"], inp["bn_mean"], inp["bn_var"])
    bnc_s, bnc_b = fold_bn(inp["bnc_gamma"], inp["bnc_beta"], inp["bnc_mean"],
                           inp["bnc_var"])
    shp = lambda a: np.ascontiguousarray(a[chs].reshape(HPC, 128).T)

    wnn1T = Wnn1[:, chs].T                                    # (512, 1024)
    wnn1 = wnn1T.reshape(HPC, 128, D).transpose(1, 0, 2).reshape(128, HPC * D)
    bnn1h = np.ascontiguousarray((0.5 * bnn1).reshape(8, 128).T)

    consts = np.concatenate([
        bq, vbias, shp(bn_s), shp(bn_b), shp(bnc_s), shp(bnc_b),
        bnn1h, np.ones((128, 2), np.float32), wspe], axis=1).astype(np.float32)
    return {
        "x_dn": _q8(x), "xpad": xpad,
        "wqkv": np.ascontiguousarray(wq.astype(F8NP)),
        "wvsum": np.ascontiguousarray(wvsum.astype(F8NP)),
        "wconv": np.ascontiguousarray(wconv8),
        "wnn": np.ascontiguousarray(wnn1.astype(ml_dtypes.bfloat16)),
        "consts": np.ascontiguousarray(consts),
        "ones8": np.ones((128, 64), F8NP),
        "identb": np.eye(128, dtype=ml_dtypes.bfloat16),
    }


_NC = None


def kernel(**inputs):
    global _NC
    if _NC is None:
        _NC = _build()
    in_maps = [_host_inputs(c, inputs) for c in range(8)]
    res = run_bass_kernel_spmd(_NC, in_maps, core_ids=list(range(8)))
    out = np.empty((B, N, D), np.float32)
    for b in range(B):
        if USE_COLLECTIVE:
            ev, od = res.results[2 * b]["out"], res.results[2 * b + 1]["out"]
            t = np.empty((D, N), np.float32)
            t[0:256] = ev[0:256]
            t[256:512] = od[0:256]
            t[512:768] = ev[256:512]
            t[768:1024] = od[256:512]
        else:
            t = res.results[2 * b]["out"] + res.results[2 * b + 1]["out"]
        out[b] = t.T
    return out


def run_timed(**inputs):
    """Re-run with NTFF tracing to get HW exec time (best effort)."""
    global _NC
    if _NC is None:
        _NC = _build()
    in_maps = [_host_inputs(c, inputs) for c in range(8)]
    try:
        return run_bass_kernel_spmd(_NC, in_maps, core_ids=list(range(8)), trace=True)
    except Exception as e:  # tracing unsupported under some axon terminals
        print(f"trace run failed: {e}")
        return None
